# revision 47
# baseline (speedup 1.0000x reference)
"""Causal attention B=4 S=4096 D=64 on 8 TRN2 NeuronCores.

Sharding: core c -> batch b = c//2, parity y = c%2. Each core computes ALL
16 q-tiles (256 rows) of its batch, but only the k-blocks of its parity:
tile i needs causal k-blocks 0..2i+1 (128 rows each); core y takes blocks
{y, y+2, ..., y+2i} = i+1 local slots. 136 blocks/core, perfectly
balanced, identical SPMD program on all cores (only DMA'd data differs:
core parity picks k/v slabs and the tri0/tri128 diagonal mask). Host
combines the two partial outputs per tile: o = (numA+numB)/(denA+denB).

Per (tile, chunk of <=4 slots): S^T[k,q] = K_blk @ Q^T via row-packed
K=64 matmul pairs run CONCURRENTLY in the two 64x128 PE row-tiles
(tile_position (0,0)/(64,0)); consecutive matmuls must hit different PSUM
banks (same-bank back-to-back new-group matmuls crash the device), hence
the EMITPOS position maps. A ~16-matmul warm-up burst on dummy data runs
during the head DMA window so the PE's HAM throttle (1.2GHz until ~3.4us
of sustained activity) is at 2.4GHz when the real stream starts. Note the
QK (64,128) <-> PV (128,128) array-mode switch drains the PE (~140ns per
QK/PV boundary); K_PV_SPLIT=1 removes it by running PV as two 64-row
tile matmuls, but costs two PSUM-evacuation copies per tile on ACT/DVE
(the only PSUM-capable engines) and measured slightly slower overall. exp(S^T/8) -> bf16 SBUF on EITHER ScalarE
(table exp) or DVE (Schraudolph: bits = A*s + B as fp32->int16, bitcast
bf16, ~2% sawtooth error) to split the exp bottleneck across two engines;
early tiles' diagonal chunks stay on exact ACT exp since short causal
windows give the diagonal a large softmax weight share. The diagonal
block is tri-masked into a separate tile so only its own PV depends on
the mask. out^T[65,256] += V'_slot.T @ P^T accumulates in PSUM (V' has a
ones column so row 64 = the softmax denominator); each tile's acc lives
in its own FULL-BANK PSUM tile so consecutive tiles' accs alternate
banks — a start=True PV after a same-bank matmul is the crash/corruption
hazard. Tiles are processed interleaved big/small (15,0),(14,1),... so
finalizes (acc copy to SBUF bf16 + DMA) spread evenly. PV emission is
deferred DEFER_PV chunks and flushed PV_BATCH at a time to limit PE mode
switches between tiled QK and full-array PV. Normalize on host in fp32.
"""

import json

import numpy as np
import ml_dtypes

import concourse.bass as bass
import concourse.mybir as mybir
import concourse.tile as tile
from concourse.bass_utils import run_bass_kernel_spmd
from concourse.vector_clock import ScopedClock

B, S, D = 4, 4096, 64
NCORES = 8
QT = 256               # q-tile width
NTILE = 16             # q-tiles per core (all tiles of the batch)
NSLOT = 16             # max k-slots per core (tile 15)
KB = 128               # k-block rows
SCALE = 1.0 / 8.0      # 1/sqrt(D)
# Schraudolph exp in bf16 bits: bits = round/trunc(A*s + B_) ; bitcast int16->bf16
A_SCHRAU = 128.0 * SCALE * 1.4426950408889634
B_SCHRAU = 16250.0

# interleave big/small tiles: pairs (15,0),(14,1),... are uniformly 17
# slots / 5 chunks, so acc-pair finalizes spread evenly through the kernel
TILE_ORDER = []
for _j in range(NTILE // 2):
    TILE_ORDER += [NTILE - 1 - _j, _j]
import os
DVE_EXTRA = int(os.environ.get("K_DVE_EXTRA", "10"))  # extra chunks on DVE exp
EXP_MODE = os.environ.get("K_EXP_MODE", "split")     # "act" | "split"
MASK_ENG = os.environ.get("K_MASK_ENG", "vector")    # "vector" | "gpsimd"
ACC_PAIR = os.environ.get("K_ACC_PAIR", "1") == "1"
DEFER_PV = int(os.environ.get("K_DEFER_PV", "5"))
DEFER_FIN = int(os.environ.get("K_DEFER_FIN", "6"))
PE_TILING = os.environ.get("K_PE_TILING", "1") == "1"  # 64x128 row tiling for QK
PV_BATCH = int(os.environ.get("K_PV_BATCH", "3"))      # PV chunks flushed together
OUT_BF16 = os.environ.get("K_OUT_BF16", "1") == "1"    # bf16 output DMA
CH = int(os.environ.get("K_CHUNK", "4"))               # slots per chunk (4 or 6)
# PV row-split: PV as two 64-row matmuls on PE row-tiles (0,0)/(64,0) keeps
# the PE in (64,128) tiling mode (mode switches drain the PE) but costs two
# PSUM-evacuation copies per tile on ACT/DVE; measured slightly behind the
# full-array PV path, so off by default.
PV_SPLIT = os.environ.get("K_PV_SPLIT", "0") == "1"
WARM_N = int(os.environ.get("K_WARM_N", "16"))         # PE warm-up matmuls
FIN_ENG = os.environ.get("K_FIN_ENG", "vg")            # finalize engines cycle
if PV_SPLIT:
    SC_BUFS = 2                                        # 4 banks sc + 4 banks acc
else:
    SC_BUFS = 3 if CH == 4 else 2                      # sc pool double/triple buffer

BF16 = mybir.dt.bfloat16
F32 = mybir.dt.float32
I16 = mybir.dt.int16
npbf16 = ml_dtypes.bfloat16


def _patched_drain_and_barrier(self, tick_clock, wait_clock):
    """The tail Drain may carry N sem waits; this walrus build rejects >1
    wait on CTRL-class instructions, so split them across N drains spread
    round-robin over all engines (a single-engine chain serializes ~100ns
    per drain; parallel chains cut the teardown tail ~5x)."""
    drain_inst = self.nc.gpsimd.drain()
    wait_clock.add_sem_waits(
        drain_inst.ins, ScopedClock({None: tick_clock.global_clock})
    )
    si = drain_inst.ins.sync_info
    waits = list(si.on_wait) if si and si.on_wait else []
    if len(waits) > 1:
        si.on_wait = waits[:1]
        engines = [self.nc.gpsimd, self.nc.scalar, self.nc.vector,
                   self.nc.tensor, self.nc.sync]
        for i, w in enumerate(waits[1:]):
            d2 = engines[i % len(engines)].drain()
            si2 = d2.ins.sync_info
            if si2 is None:
                d2.ins.sync_info = mybir.SyncInfo(on_wait=[w], on_update=[])
            else:
                si2.on_wait = [w]
    popped = self.nc._tile_sem_poison_stack.pop()
    assert popped is self._sem_poison
    self.nc.clear_and_free_semaphores(list(self.sems.allocated().values()))


tile.TileContext._drain_and_barrier = _patched_drain_and_barrier

if hasattr(bass.Bass, "_orig_to_json_bytes"):
    _orig_to_json_bytes = bass.Bass._orig_to_json_bytes
else:
    _orig_to_json_bytes = bass.Bass.to_json_bytes
    bass.Bass._orig_to_json_bytes = _orig_to_json_bytes


def _to_json_bytes_split_waits(self) -> bytes:
    """This walrus build accepts at most one sem wait per instruction; spill
    extra waits onto standalone EventSemaphore instructions just before."""
    m = json.loads(_orig_to_json_bytes(self))
    ctr = 0
    for fn in m["functions"]:
        for blk in fn["blocks"]:
            if blk["name"] == "main":
                blk["instructions"] = [
                    i for i in blk["instructions"]
                    if i["opcode"] not in ("Memset", "Drain", "EventSemaphore")
                ]
        for blk in fn["blocks"]:
            out = []
            for inst in blk["instructions"]:
                si = inst.get("sync_info")
                ow = (si or {}).get("on_wait") or []
                if ow:
                    # engines execute their queue in order, so a wait on the
                    # instruction's own engine-completion semaphore is always
                    # already satisfied — drop it (DMA queue sems are named
                    # DMAHW*/DMASW* and never match the engine string).
                    eng = inst.get("engine")
                    kept = [
                        w for w in ow
                        if w.get("ant_name", "").rsplit("_", 1)[0] != eng
                    ]
                    if len(kept) != len(ow):
                        si["on_wait"] = ow = kept
                if len(ow) > 1:
                    for w in ow[:-1]:
                        ctr += 1
                        out.append({
                            "debug": inst.get("debug", 0),
                            "engine": inst["engine"],
                            "ins": [],
                            "outs": [],
                            "name": f"{inst['name']}_sw{ctr}",
                            "opcode": "EventSemaphore",
                            "sync_info": {"on_update": [], "on_wait": [w]},
                        })
                    si["on_wait"] = [ow[-1]]
                out.append(inst)
            blk["instructions"] = out
    return json.dumps(m).encode()


bass.Bass.to_json_bytes = _to_json_bytes_split_waits


def _chunk_list():
    """Processing order: [(tile_i, proc_pos, s0, ln, is_last_chunk)]."""
    out = []
    for k, i in enumerate(TILE_ORDER):
        n = i + 1
        for s0 in range(0, n, CH):
            ln = min(CH, n - s0)
            out.append((i, k, s0, ln, s0 + ln == n))
    return out


def _dve_chunk_ids(chunks):
    """Chunk indices whose exp runs on DVE (approximate Schraudolph exp).

    Early tiles (i < 8) have short causal windows, so their diagonal block
    carries a big softmax weight share — keep those on exact ACT exp.
    DVE gets the deep-window diagonal chunks (tiles >= 8) plus DVE_EXTRA
    full chunks spread evenly, where per-element error averages out."""
    ids = {ci for ci, (i, _, _, _, last) in enumerate(chunks)
           if last and i >= 8}
    nonlast = [ci for ci, (_, _, _, _, last) in enumerate(chunks) if not last]
    if DVE_EXTRA > 0 and nonlast:
        step = max(1, len(nonlast) // DVE_EXTRA)
        picked = 0
        for j, ci in enumerate(nonlast):
            if j % step == step // 2 and picked < DVE_EXTRA:
                ids.add(ci)
                picked += 1
    return ids


def build_nc():
    chunks = _chunk_list()
    dve_ids = _dve_chunk_ids(chunks)

    nc = bass.Bass()
    qT_d = nc.declare_dram_parameter("qT", [128, NTILE * QT], BF16, isOutput=False)
    kT_d = nc.declare_dram_parameter("kT", [128, 8 * KB], BF16, isOutput=False)
    v_d = nc.declare_dram_parameter("v", [128, NSLOT * 65], BF16, isOutput=False)
    m_d = nc.declare_dram_parameter("dmask", [128, QT], BF16, isOutput=False)
    out_dt = BF16 if OUT_BF16 else F32
    if PV_SPLIT:
        # both row-tile partial accs ship to HBM; host sums the halves
        out_d = nc.declare_dram_parameter("outT", [65, NTILE, 2, QT], out_dt,
                                          isOutput=True)
    else:
        out_d = nc.declare_dram_parameter("outT", [65, NTILE, QT], out_dt,
                                          isOutput=True)

    with tile.TileContext(nc, pool_alloc_mode="queue") as tc:
        with (
            tc.tile_pool(name="const", bufs=1) as cpool,
            tc.tile_pool(name="pTa", bufs=DEFER_PV + 1) as apool,
            tc.tile_pool(name="pTv", bufs=DEFER_PV + 1) as vpool,
            tc.tile_pool(name="pTm", bufs=3) as mpool,
            tc.tile_pool(name="osb", bufs=2) as opool,
            tc.tile_pool(name="sc", bufs=SC_BUFS, space="PSUM") as scpool,
            tc.tile_pool(name="acc", bufs=2, space="PSUM") as accpool,
        ):
            # zero bias for ACT + warm the exp table during the DMA window;
            # memsets on gpsimd so the ACT's table load isn't gated on the
            # (busier) vector queue
            zbias = cpool.tile([128, 1], F32)
            nc.gpsimd.memset(zbias[:], 0.0)
            warm = cpool.tile([128, 8], F32)
            nc.gpsimd.memset(warm[:], 0.0)
            nc.scalar.activation(out=warm[:], in_=warm[:], bias=zbias[:],
                                 func=mybir.ActivationFunctionType.Exp)

            # PE warm-up: the HAM throttle runs the PE at 1.2GHz until it has
            # been continuously busy for a free-running ~3.4us window. Burn
            # that window on dummy (64,128)-mode matmuls during the head DMA
            # wait so the real stream starts at 2.4GHz. Same tile config as
            # the real matmuls -> no mode-switch drain at the transition.
            if WARM_N > 0:
                wrm_w = cpool.tile([128, QT], BF16)
                nc.gpsimd.memset(wrm_w[:], 0.0)
                wps = scpool.tile([128, CH, QT], F32, name="sc")
                for wi in range(WARM_N):
                    lo = wi % 2 == 0
                    rows = slice(0, 64) if lo else slice(64, 128)
                    kw = {"tile_position": (0, 0) if lo else (64, 0)}
                    nc.tensor.matmul(wps[:, wi % 2 * 2, :],
                                     wrm_w[rows, 0:128], wrm_w[rows],
                                     start=True, stop=True, **kw)

            # one tile per DMA chunk so consumers wait only on their chunk
            if CH == 4:
                KSPLIT = (2, 4, 8)
                VSPLIT = (4, 8, 16)
            else:
                KSPLIT = (3, 6, 8)
                VSPLIT = (6, 12, 16)
            qT_a = cpool.tile([128, 1, QT], BF16)    # tile pos 0 (= tile 15)
            qT_b = cpool.tile([128, 3, QT], BF16)    # pos 1..3
            qT_c = cpool.tile([128, 12, QT], BF16)   # pos 4..15
            kT_a = cpool.tile([128, KSPLIT[0], KB], BF16)
            kT_b = cpool.tile([128, KSPLIT[1] - KSPLIT[0], KB], BF16)
            kT_c = cpool.tile([128, KSPLIT[2] - KSPLIT[1], KB], BF16)
            v_a = cpool.tile([128, VSPLIT[0], 65], BF16)
            v_b = cpool.tile([128, VSPLIT[1] - VSPLIT[0], 65], BF16)
            v_c = cpool.tile([128, VSPLIT[2] - VSPLIT[1], 65], BF16)
            mask_sb = cpool.tile([128, QT], BF16)

            qT_dr = qT_d.rearrange("p (s q) -> p s q", q=QT)
            kT_dr = kT_d.rearrange("p (b k) -> p b k", k=KB)
            v_dr = v_d.rearrange("p (b v) -> p b v", v=65)

            # head: first-needed chunks on parallel queues. kT_a on scalar
            # (first QK needs it ASAP, before the exp stream exists); the
            # rest of kT on the tensor queue, which sits idle until the
            # first LDWEIGHTS -- this keeps the ACT table-load + exp stream
            # unblocked on scalar.
            kq2 = nc.sync if PV_SPLIT else nc.scalar
            nc.sync.dma_start(out=qT_a[:], in_=qT_dr[:, 0:1])
            nc.scalar.dma_start(out=kT_a[:], in_=kT_dr[:, 0:KSPLIT[0]])
            nc.gpsimd.dma_start(out=v_a[:], in_=v_dr[:, 0:VSPLIT[0]])
            kq2.dma_start(out=kT_b[:], in_=kT_dr[:, KSPLIT[0]:KSPLIT[1]])
            nc.sync.dma_start(out=qT_b[:], in_=qT_dr[:, 1:4])
            kq2.dma_start(out=kT_c[:], in_=kT_dr[:, KSPLIT[1]:KSPLIT[2]])
            nc.gpsimd.dma_start(out=v_b[:], in_=v_dr[:, VSPLIT[0]:VSPLIT[1]])
            nc.gpsimd.dma_start(out=v_c[:], in_=v_dr[:, VSPLIT[1]:VSPLIT[2]])
            nc.sync.dma_start(out=qT_c[:], in_=qT_dr[:, 4:16])
            nc.gpsimd.dma_start(out=mask_sb[:], in_=m_d[:])

            def q_at(pos):
                if pos == 0:
                    return qT_a[:, 0, :]
                if pos < 4:
                    return qT_b[:, pos - 1, :]
                return qT_c[:, pos - 4, :]

            def kT_at(pair):
                if pair < KSPLIT[0]:
                    return kT_a[:, pair, :]
                if pair < KSPLIT[1]:
                    return kT_b[:, pair - KSPLIT[0], :]
                return kT_c[:, pair - KSPLIT[1], :]

            def v_at(slot):
                if slot < VSPLIT[0]:
                    return v_a[:, slot, :]
                if slot < VSPLIT[1]:
                    return v_b[:, slot - VSPLIT[0], :]
                return v_c[:, slot - VSPLIT[1], :]

            # sc positions per chunk length, in emission order: consecutive
            # matmuls must target different PSUM banks (positions 2k, 2k+1
            # share bank k) — same-bank back-to-back new-group matmuls crash
            # the device.
            EMITPOS = {6: (0, 2, 4, 1, 3, 5), 5: (0, 2, 4, 1, 3),
                       4: (0, 2, 1, 3), 3: (0, 2, 1), 2: (1, 2), 1: (0,)}
            EXPLO = {6: 0, 5: 0, 4: 0, 3: 0, 2: 1, 1: 0}

            # per-tile acc handling: tiles paired by processing position
            acc_pairs = {}

            def acc_for(pos):
                # one acc per tile; pool ring (bufs=2) inserts the WAR deps.
                # PV_SPLIT: [65, 4, QT] = 2 banks; col 0 = lo half (bank a),
                # col 2 = hi half (bank b) -- concurrent T0/T8 row-tile
                # matmuls must write different banks.
                if pos not in acc_pairs:
                    if PV_SPLIT:
                        acc_pairs[pos] = accpool.tile([65, 4, QT], F32,
                                                      name="accp")
                    else:
                        acc_pairs[pos] = accpool.tile([65, 2 * QT], F32,
                                                      name="accp")
                if PV_SPLIT:
                    return acc_pairs[pos]
                return acc_pairs[pos][:, 0:QT]

            def emit_qk(ci):
                i, pos, s0, ln, last = chunks[ci]
                sc = scpool.tile([128, CH, QT], F32)
                qt = q_at(pos)
                p0 = s0 // 2
                for off in range(ln):
                    kt = kT_at(p0 + off // 2)
                    lo = off % 2 == 0
                    rows = slice(0, 64) if lo else slice(64, 128)
                    kw = {}
                    if PE_TILING:
                        kw["tile_position"] = (0, 0) if lo else (64, 0)
                    nc.tensor.matmul(sc[:, EMITPOS[ln][off], :], kt[rows],
                                     qt[rows], start=True, stop=True, **kw)
                return sc

            exp_ramp_n = int(os.environ.get("K_EXP_RAMP", "2"))

            def emit_exp(ci, sc):
                i, pos, s0, ln, last = chunks[ci]
                lo = EXPLO[ln]
                hi = lo + ln
                if ci < exp_ramp_n and ln >= 2:
                    # ramp: run this chunk's exp on BOTH engines in parallel
                    # (ACT on the first half, DVE Schraudolph on the second)
                    # to halve the first exp latency -- the PE idles ~2us
                    # early on waiting for the first P and sc recycle.
                    mid = lo + ln // 2
                    pT = apool.tile([128, CH, QT], BF16)
                    nc.scalar.activation(
                        out=pT[:, lo:mid, :], in_=sc[:, lo:mid, :],
                        bias=zbias[:],
                        func=mybir.ActivationFunctionType.Exp, scale=SCALE,
                    )
                    pTi = vpool.tile([128, CH, QT], I16)
                    nc.vector.tensor_scalar(
                        out=pTi[:, mid:hi, :], in0=sc[:, mid:hi, :],
                        scalar1=A_SCHRAU, scalar2=B_SCHRAU,
                        op0=mybir.AluOpType.mult, op1=mybir.AluOpType.add,
                    )
                    pTi_b = pTi[:].bitcast(BF16)
                    # stitch: PV reads per-position slices; positions < mid
                    # come from pT, >= mid from the bitcast tile. Return a
                    # picker instead of a single AP.
                    srcs = [pT[:] if p < mid else pTi_b
                            for p in range(CH)]
                    if last:
                        mp = EMITPOS[ln][ln - 1]
                        mo = mpool.tile([128, QT], BF16, name="maskout")
                        meng = nc.gpsimd if (
                            MASK_ENG == "gpsimd" and pos < NTILE - 2
                        ) else nc.vector
                        meng.tensor_mul(mo[:], srcs[mp][:, mp, :], mask_sb[:])
                        return srcs, mo
                    return srcs, None
                if EXP_MODE == "split" and ci in dve_ids:
                    pT = vpool.tile([128, CH, QT], I16)
                    nc.vector.tensor_scalar(
                        out=pT[:, lo:hi, :], in0=sc[:, lo:hi, :],
                        scalar1=A_SCHRAU, scalar2=B_SCHRAU,
                        op0=mybir.AluOpType.mult, op1=mybir.AluOpType.add,
                    )
                    pT_b = pT[:].bitcast(BF16)
                else:
                    pT = apool.tile([128, CH, QT], BF16)
                    nc.scalar.activation(
                        out=pT[:, lo:hi, :], in_=sc[:, lo:hi, :], bias=zbias[:],
                        func=mybir.ActivationFunctionType.Exp, scale=SCALE,
                    )
                    pT_b = pT[:]
                if last:
                    # diagonal block: position of chunk-local offset ln-1.
                    # Mask into a separate tile so only the diagonal PV
                    # depends on the mask op (deps are tile-granular).
                    # gpsimd is ~3x slower per element, so the final tiles'
                    # masks (tail critical path) go on vector instead.
                    mp = EMITPOS[ln][ln - 1]
                    mo = mpool.tile([128, QT], BF16, name="maskout")
                    meng = nc.gpsimd if (
                        MASK_ENG == "gpsimd" and pos < NTILE - 2
                    ) else nc.vector
                    meng.tensor_mul(mo[:], pT_b[:, mp, :], mask_sb[:])
                    return pT_b, mo
                return pT_b, None

            def emit_pv(ci, pT_b, mo):
                i, pos, s0, ln, last = chunks[ci]
                acc = acc_for(pos)
                n = i + 1
                pick = (lambda p: pT_b[p]) if isinstance(pT_b, list) \
                    else (lambda p: pT_b)
                for off in range(ln):
                    slot = s0 + off
                    diag = last and off == ln - 1
                    if PV_SPLIT:
                        # two 64-row matmuls on PE row-tiles: stays in the
                        # same (64,128) array mode as QK (no PE drain), and
                        # the lo/hi streams run concurrently. hi first so the
                        # bank sequence alternates ...lo(b_a) hi(b_b) and a
                        # start=True never follows its own bank.
                        mp = EMITPOS[ln][off]
                        src_lo = mo[0:64, :] if diag else pick(mp)[0:64, mp, :]
                        src_hi = mo[64:128, :] if diag \
                            else pick(mp)[64:128, mp, :]
                        vs = v_at(slot)
                        nc.tensor.matmul(
                            acc[:, 2, :], vs[64:128], src_hi,
                            start=(slot == 0), stop=(slot == n - 1),
                            tile_position=(64, 0),
                        )
                        nc.tensor.matmul(
                            acc[:, 0, :], vs[0:64], src_lo,
                            start=(slot == 0), stop=(slot == n - 1),
                            tile_position=(0, 0),
                        )
                    else:
                        mp = EMITPOS[ln][off]
                        src = mo[:] if diag else pick(mp)[:, mp, :]
                        nc.tensor.matmul(
                            acc, v_at(slot), src,
                            start=(slot == 0), stop=(slot == n - 1),
                        )

            copy_cfg = os.environ.get("K_COPY", "sv")
            if copy_cfg == "vec":
                COPY_ENG = (nc.vector, nc.vector)
            else:
                COPY_ENG = (nc.scalar, nc.vector)

            def emit_finalize(pos):
                acc = acc_pairs.pop(pos)
                if PV_SPLIT:
                    # evacuate both partial accs with ONE strided copy (single
                    # PSUM source spanning both banks); alternate ACT/DVE (the
                    # only PSUM-capable engines). Host sums the halves.
                    o_sb = opool.tile([65, 2, QT], out_dt, name="o_sb")
                    a, b = (0, 2) if pos % 2 == 0 else (2, 0)
                    nc.scalar.activation(
                        out=o_sb[:, a // 2, :], in_=acc[:, a, :], bias=0.0,
                        func=mybir.ActivationFunctionType.Copy,
                    )
                    nc.vector.tensor_copy(o_sb[:, b // 2, :], acc[:, b, :])
                    dq = nc.sync if pos % 2 == 0 else nc.gpsimd
                    dq.dma_start(out=out_d[:, pos, :, :], in_=o_sb[:])
                    return
                o_sb = opool.tile([65, 1, QT], out_dt, name="o_sb")
                eng = COPY_ENG[pos % 2]
                if eng is nc.scalar:
                    nc.scalar.activation(
                        out=o_sb[:, 0, :], in_=acc[:, 0:QT], bias=0.0,
                        func=mybir.ActivationFunctionType.Copy,
                    )
                else:
                    nc.vector.tensor_copy(o_sb[:, 0, :], acc[:, 0:QT])
                dq = nc.sync if pos % 2 == 0 else nc.gpsimd
                dq.dma_start(
                    out=out_d[:, pos:pos + 1, :], in_=o_sb[:]
                )

            def emit_warm2(nmm):
                # hole-filler matmuls into the first two acc tiles: the PE
                # would otherwise idle ~2us after the first QK chunks while
                # the first exp drains sc. The first real PV's start=True
                # resets the PSUM, so the garbage written here is free.
                # Alternating acc tiles = alternating banks.
                if WARM_N <= 0 or nmm <= 0:
                    return
                a0, a1 = acc_for(0), acc_for(1)
                if PV_SPLIT:
                    t0, t1 = a0[:, 0, :], a1[:, 2, :]
                else:
                    t0, t1 = a0, a1
                for wi in range(nmm):
                    lo = wi % 2 == 0
                    rows = slice(0, 64) if lo else slice(64, 128)
                    nc.tensor.matmul(
                        t0 if lo else t1, wrm_w[rows, 0:65], wrm_w[rows],
                        start=True, stop=True,
                        tile_position=(0, 0) if lo else (64, 0),
                    )

            pending_pv = []    # (ci, pT_b) awaiting PV emission
            pending_fin = []   # (due_ci, pos) awaiting finalize emission

            nchunks = min(len(chunks), int(os.environ.get("K_MAX_CHUNKS", "999")))
            qk_only_from = int(os.environ.get("K_QK_ONLY_FROM", "9999"))
            warm2 = int(os.environ.get("K_WARM2", "0"))
            for ci in range(nchunks):
                i, pos, s0, ln, last = chunks[ci]
                sc = emit_qk(ci)
                if ci == 2:
                    emit_warm2(warm2)
                if ci >= qk_only_from:
                    continue
                pT_b, mo = emit_exp(ci, sc)
                while pending_fin and pending_fin[0][0] <= ci:
                    fpos = pending_fin.pop(0)[1]
                    # the copy must come after every PV of its tile; optionally
                    # drain the whole queue so PV bursts stay consolidated
                    # (fewer QK<->PV array-mode transitions)
                    flush_all = os.environ.get("K_FIN_FLUSH", "pos") == "all"
                    while pending_pv and (flush_all or
                                          chunks[pending_pv[0][0]][1] <= fpos):
                        cj, pj, mj = pending_pv.pop(0)
                        emit_pv(cj, pj, mj)
                    emit_finalize(fpos)
                if len(pending_pv) >= DEFER_PV + PV_BATCH - 1:
                    for _ in range(PV_BATCH):
                        cj, pj, mj = pending_pv.pop(0)
                        emit_pv(cj, pj, mj)
                pending_pv.append((ci, pT_b, mo))
                if last:
                    # acc pool is double-buffered: the copy of tile pos must
                    # be emitted before tile pos+2's first PV hits the PE
                    # queue
                    due = ci + DEFER_FIN
                    nxt = [cj for cj, ch in enumerate(chunks)
                           if ch[1] == pos + 2]
                    if nxt:
                        due = min(due, nxt[0] + DEFER_PV - 1)
                    pending_fin.append((due, pos))
            for cj, pj, mj in pending_pv:
                emit_pv(cj, pj, mj)
            for _, pos in pending_fin:
                emit_finalize(pos)
    return nc


def _host_inputs(q, k, v):
    """Build per-core device input maps. q,k,v: [B,S,D] float32."""
    r = np.arange(KB)[:, None]
    j = np.arange(QT)[None, :]
    tri = [(j >= r).astype(npbf16), (j >= r + KB).astype(npbf16)]

    in_maps = []
    for c in range(NCORES):
        b, y = c // 2, c % 2
        qb = q[b].astype(npbf16)
        kb = k[b].astype(npbf16)
        vb = v[b].astype(npbf16)
        # qT: [128, 16*256], processing pos p = tile TILE_ORDER[p], q-block
        # transposed and duplicated in both partition halves
        qT = np.empty((128, NTILE * QT), npbf16)
        for p, i in enumerate(TILE_ORDER):
            blk = qb[i * QT:(i + 1) * QT, :].T  # [64, 256]
            qT[0:64, p * QT:(p + 1) * QT] = blk
            qT[64:128, p * QT:(p + 1) * QT] = blk
        # local slot s -> global k-block y + 2s; kT pair p packs slots
        # (2p, 2p+1) in partition halves
        kslab = kb.reshape(32, KB, D)[y::2]          # [16, 128, 64]
        kT = kslab.reshape(8, 2, KB, D).transpose(1, 3, 0, 2).reshape(128, 8 * KB)
        # v': [128, 16*65] slot-major with ones column
        va = np.concatenate([vb, np.ones((S, 1), npbf16)], axis=1)
        vdev = va.reshape(32, KB, 65)[y::2].transpose(1, 0, 2).reshape(128, NSLOT * 65)
        in_maps.append({
            "qT": np.ascontiguousarray(qT),
            "kT": np.ascontiguousarray(kT),
            "v": np.ascontiguousarray(vdev),
            "dmask": np.ascontiguousarray(tri[y]),
        })
    return in_maps


_LAST_PERF = {}


def kernel(q, k, v, causal, trace=False):
    q = np.asarray(q, np.float32)
    k = np.asarray(k, np.float32)
    v = np.asarray(v, np.float32)
    if int(causal) != 1:
        # non-causal fallback (not the optimized path)
        sim = np.einsum("bqd,bkd->bqk", q, k) / np.sqrt(np.float32(D))
        sim -= sim.max(axis=2, keepdims=True)
        p = np.exp(sim)
        p /= p.sum(axis=2, keepdims=True)
        return np.einsum("bqk,bkd->bqd", p, v).astype(np.float32)
    nc = build_nc()
    in_maps = _host_inputs(q, k, v)
    res = run_bass_kernel_spmd(nc, in_maps, core_ids=list(range(NCORES)), trace=trace)
    _LAST_PERF["exec_time_ns"] = res.exec_time_ns
    _LAST_PERF["trace"] = res.instructions_and_trace
    _LAST_PERF["mean_exec_time_ns"] = res.mean_exec_time_ns

    out = np.empty((B, S, D), np.float32)
    for b in range(B):
        oA = res.results[2 * b]["outT"].astype(np.float32)      # parity 0
        oB = res.results[2 * b + 1]["outT"].astype(np.float32)  # parity 1
        if PV_SPLIT:
            oA = oA.sum(axis=2)   # [65, NTILE, 2, QT] -> lo+hi halves
            oB = oB.sum(axis=2)
        for p, i in enumerate(TILE_ORDER):
            num = oA[0:64, p, :] + oB[0:64, p, :]
            den = oA[64, p, :] + oB[64, p, :]
            out[b, i * QT:(i + 1) * QT, :] = (num / den).T
    return out

